# revision 80
# baseline (speedup 1.0000x reference)
"""Distributed Trainium2 Bass kernel for causal multi-head attention.

Problem: B=2, T=2048, C=1024, H=16 heads (Dh=64), RoPE + causal mask +
softmax + output projection.

Sharding: 8 cores = batch (2) x head-groups (4 heads each). Each core
computes q/k/v projections for its 4 heads, RoPE, attention, and a
partial output projection y_partial = out_heads @ Wo_slice.T. The host
sums the 4 partials per batch element.

Layout: everything is computed in "head-dim-major" (transposed) layout
so no on-chip transposes are needed. q and k are stored as head-pair
tiles (h0 dims on partitions 0-63, h1 on 64-127).

PE array tiling (the core trick): with Dh=64, attention matmuls only
use half the 128-wide contraction/output. The PE is therefore run in
tiled mode:
  QK: 64x128 row tiling - both heads of a pair score one key tile in a
      single 512-col span (T0 streams h0 from partitions 0-63, T8
      streams h1 from 64-127; outputs land in different PSUM banks).
  PV: 128x64 column tiling - both heads' O^T accumulate side by side in
      one PSUM bank (T0 -> partitions 0-63, T1 -> 64-127).
  softmax denominators: 128x32 column tiling - all 4 heads' ones-row
      reductions of E run as one concurrent quad per key tile, batched
      at chunk end.
Measured on HW: each tile of a span runs concurrently (~108ns per
64-tile 512-col matmul vs ~228ns untiled), so attention PE time halves.

Schedule: x is DMA'd in two token-halves (2KB partition rows) with the
qkproj-0 constants interleaved so attention starts early; later qkproj
parts, vproj tiles, and the previous chunk's o-projection are emitted
at attention group boundaries as PE filler under the scalar-engine exp
(which is the attention-phase bottleneck). The TileContext scheduler
dispatches per-engine by readiness with emission-order priority, so
these fillers actually fill PE stalls.
"""

import os
import sys
import types
import numpy as np

sys.path.insert(0, "/opt/trn_rl_repo")

import ml_dtypes
import concourse.bass as bass
import concourse.mybir as mybir
from concourse import bacc
from concourse.tile import TileContext
from concourse import bass_utils
from concourse.bass import ts, ds

F32 = mybir.dt.float32
BF16 = mybir.dt.bfloat16

B, T, C, H = 2, 2048, 1024, 16
Dh = C // H          # 64
HG = 4               # heads per core
NCORES = 8
KC = C // 128        # 8 contraction tiles for projections
NCHUNK = T // 512    # 4 token chunks
KT = T // 128        # 16 key tiles
SCALE = Dh ** -0.5   # 0.125


def _install_ntff_hook():
    """The NTFF profiling hook module is absent in this image; inject it."""
    if "antenv.axon_hooks" in sys.modules:
        return
    try:
        import trn_agent_boot.trn_boot as tb
        mod = types.ModuleType("antenv.axon_hooks")
        hook = tb._ntff_profile_via_ctypes("/opt/axon/libaxon_pjrt.so")
        mod.get_axon_ntff_profile_hook = lambda: hook
        sys.modules["antenv.axon_hooks"] = mod
    except Exception:
        pass


def build(mode: str, dbg: bool = False) -> bass.Bass:
    """mode: 'causal' | 'full' | 'general'"""
    assert mode in ("causal", "full", "general")
    nc = bacc.Bacc(None, target_bir_lowering=False)
    if dbg:
        d_qT0 = nc.dram_tensor("d_qT0", [128, T], BF16, kind="ExternalOutput")
        d_kT0 = nc.dram_tensor("d_kT0", [128, T], BF16, kind="ExternalOutput")
        d_v0 = nc.dram_tensor("d_v0", [128, 256], BF16, kind="ExternalOutput")
        d_v5 = nc.dram_tensor("d_v5", [128, 256], BF16, kind="ExternalOutput")
        d_dD = nc.dram_tensor("d_dD", [128, 512], BF16, kind="ExternalOutput")
        d_E = nc.dram_tensor("d_E", [128, 1024], BF16, kind="ExternalOutput")
        d_o0 = nc.dram_tensor("d_o0", [128, 512], BF16, kind="ExternalOutput")
        d_psO0 = nc.dram_tensor("d_psO0", [128, 512], F32,
                                kind="ExternalOutput")
        d_rb0 = nc.dram_tensor("d_rb0", [128, 512], F32, kind="ExternalOutput")
        d_ri0 = nc.dram_tensor("d_ri0", [128, 512], F32, kind="ExternalOutput")

    xT = nc.dram_tensor("xT", [C, T], BF16, kind="ExternalInput")
    wq = nc.dram_tensor("wq", [C, 256], BF16, kind="ExternalInput")
    wk = nc.dram_tensor("wk", [C, 256], BF16, kind="ExternalInput")
    wv = nc.dram_tensor("wv", [C, 256], BF16, kind="ExternalInput")
    wo = nc.dram_tensor("wo", [256, C], BF16, kind="ExternalInput")
    cos2 = nc.dram_tensor("cos2", [128, T], BF16, kind="ExternalInput")
    sin2 = nc.dram_tensor("sin2", [128, T], BF16, kind="ExternalInput")
    tri = nc.dram_tensor("tri", [128, 128], BF16, kind="ExternalInput")
    # blk rows {0,32}: [1]*64+[0]*64, rows {1,33}: [0]*64+[1]*64 — used to
    # replicate per-head softmax denominators across their 64 psO partitions
    blk = nc.dram_tensor("blk", [128, 128], BF16, kind="ExternalInput")
    if mode == "general":
        maskT = nc.dram_tensor("maskT", [T, T], BF16, kind="ExternalInput")
    y = nc.dram_tensor("out", [T, C], BF16, kind="ExternalOutput")

    with TileContext(nc) as tc:
        with (
            tc.tile_pool(name="persist", bufs=1) as persist,
            tc.tile_pool(name="epool", bufs=8) as epool,
            tc.tile_pool(name="rope", bufs=4) as rope,
            tc.tile_pool(name="opool", bufs=2) as opool,
            tc.tile_pool(name="psum", bufs=2, space="PSUM") as psum,
            tc.tile_pool(name="mpool", bufs=1) as mpool,
        ):
            # ---- persistent SBUF tensors ----
            qT_sb = [persist.tile([128, T], BF16, name=f"qT{p}") for p in range(2)]
            kT_sb = [persist.tile([128, T], BF16, name=f"kT{p}") for p in range(2)]
            # v token-major, pair-contiguous: cols = h0|h1|h2|h3 dims
            v_sb = [persist.tile([128, HG * Dh], BF16, name=f"v{j}")
                    for j in range(KT)]
            wo_sb = persist.tile([128, 2, C], BF16, name="wo_sb")
            tri_sb = persist.tile([128, 128], BF16, name="tri_sb")
            blk_sb = persist.tile([128, 128], BF16, name="blk_sb")
            ones_sb = persist.tile([128, 1], BF16, name="ones_sb")
            nc.vector.memset(ones_sb[:], 1.0)
            dT_sb = persist.tile([128, 512], BF16, name="dT_sb")
            nc.vector.memset(dT_sb[:], 0.0)
            # x staged in two token-halves: 2KB partition rows keep DMA
            # throughput high while the first half still lands early
            xh_sb = [persist.tile([128, KC, T // 2], BF16, name=f"x{n}")
                     for n in range(2)]
            xv = xT.rearrange("(kt p) t -> kt p t", p=128)
            w_sb = {}
            for nm in ("v", "q", "k"):
                w_sb[nm] = persist.tile([128, KC, 256], BF16, name=f"w{nm}_sb")
            # cos/sin ship as bf16 (half the DMA) and are cast once to fp32
            # on-chip: the RoPE multiplies need fp32 operands for DVE speed
            cosb_sb = persist.tile([128, T], BF16, name="cosb_sb")
            sinb_sb = persist.tile([128, T], BF16, name="sinb_sb")
            cos_sb = persist.tile([128, T], F32, name="cos_sb")
            sin_sb = persist.tile([128, T], F32, name="sin_sb")

            # input DMA: qkproj-0 deps (wq/wk/cos/sin) and vproj deps (wv,
            # x chunk 0) split across the two fast hwdge rings; the slow
            # gpsimd software ring only carries later x chunks.
            for k in range(KC):
                (nc.sync if k % 2 == 0 else nc.scalar).dma_start(
                    xh_sb[0][:, k, :], xv[k][:, ts(0, 1024)])
            nc.sync.dma_start(w_sb["q"][:],
                              wq.rearrange("(kt p) m -> p kt m", p=128))
            nc.scalar.dma_start(w_sb["k"][:],
                                wk.rearrange("(kt p) m -> p kt m", p=128))
            nc.sync.dma_start(cosb_sb[:], cos2[:])
            nc.scalar.dma_start(sinb_sb[:], sin2[:])
            nc.scalar.dma_start(w_sb["v"][:],
                                wv.rearrange("(kt p) m -> p kt m", p=128))
            nc.vector.tensor_copy(cos_sb[:], cosb_sb[:])
            nc.vector.tensor_copy(sin_sb[:], sinb_sb[:])
            nc.sync.dma_start(tri_sb[:], tri[:])
            nc.sync.dma_start(blk_sb[:], blk[:])
            for k in range(KC):
                (nc.sync if k % 2 == 0 else nc.scalar).dma_start(
                    xh_sb[1][:, k, :], xv[k][:, ts(1, 1024)])
            nc.scalar.dma_start(wo_sb[:],
                                wo.rearrange("(p2 p) n -> p p2 n", p=128))
            if mode == "general":
                mv = maskT.rearrange("(kt p) t -> kt p t", p=128)

            # ---------------- emission helpers ----------------
            def emit_vproj(tt):
                ps = psum.tile([128, 512], F32, tag="proj" if tt % 2 else "oy",
                               bufs=1, name="psv")
                for k in range(KC):
                    nc.tensor.matmul(
                        ps[:, 0:256],
                        xh_sb[tt // 8][:, k, ds(128 * (tt % 8), 128)],
                        w_sb["v"][:, k, :],
                        start=(k == 0), stop=(k == KC - 1))
                nc.vector.tensor_copy(v_sb[tt][:], ps[:, 0:256])

            def emit_qkproj(n, part):
                # part: 0=q pair0, 1=q pair1, 2=k pair0, 3=k pair1
                nm = "q" if part < 2 else "k"
                dest = qT_sb if part < 2 else kT_sb
                p = part % 2
                ps = psum.tile([128, 512], F32,
                               tag="proj" if part % 2 else "oy",
                               bufs=1, name="psp")
                for k in range(KC):
                    nc.tensor.matmul(
                        ps[:],
                        w_sb[nm][:, k, ts(p, 128)],
                        xh_sb[n // 2][:, k, ds(512 * (n % 2), 512)],
                        start=(k == 0), stop=(k == KC - 1))
                # RoPE: out = ps*cos + swap32(ps * sin_preswapped)
                t1 = rope.tile([128, 512], BF16, tag="t1", name="t1")
                nc.vector.tensor_mul(t1[:], ps[:], cos_sb[:, ts(n, 512)])
                t2p = rope.tile([128, 512], BF16, tag="t2p", name="t2p")
                nc.vector.tensor_mul(t2p[:], ps[:], sin_sb[:, ts(n, 512)])
                t2 = rope.tile([128, 512], BF16, tag="t2", name="t2")
                for a, bq in ((0, 32), (32, 0), (64, 96), (96, 64)):
                    nc.sync.dma_start(t2[a:a + 32, :], t2p[bq:bq + 32, :])
                nc.vector.tensor_add(dest[p][:, ts(n, 512)], t1[:], t2[:])

            def emit_oproj_unit(o_pairs, tt, tags=("oy", "oy"), split=False):
                y_sb = opool.tile([128, C], BF16, tag="y", bufs=3, name="y_sb")
                for nn in range(2):
                    psY = psum.tile([128, 512], F32, tag=tags[nn], bufs=1,
                                    name="psY")
                    for p in range(2):
                        nc.tensor.matmul(
                            psY[:],
                            o_pairs[1][p][:, ts(tt, 128)],
                            wo_sb[:, p, ts(nn, 512)],
                            start=(p == 0), stop=(p == 1))
                    nc.vector.tensor_copy(y_sb[:, ts(nn, 512)], psY[:])
                c = o_pairs[0]
                nc.sync.dma_start(y[ds(512 * c + 128 * tt, 128), :], y_sb[:])

            def n_off_of(c, j):
                if mode == "causal" and j >= 4 * c:
                    return 128 * (j - 4 * c)
                return 0

            def emit_attn(c, fillers, last=False):
                """attention for chunk c; fillers emitted at group ends"""
                nkt = 4 * (c + 1) if mode == "causal" else KT
                ngroups = nkt // 2
                if mode == "general":
                    msk_sb = mpool.tile([128, KT, 512], BF16, bufs=2,
                                        name="msk_sb")
                    for j in range(KT):
                        nc.sync.dma_start(msk_sb[:, j, :], mv[j][:, ts(c, 512)])
                psO = [psum.tile([128, 512], F32, tag=f"o{hp}", bufs=1,
                                 name=f"psO{hp}") for hp in range(2)]
                Es = {}
                nfill = len(fillers)
                nemit = 0
                pair_i = 0
                for g in range(ngroups):
                    js = (2 * g, 2 * g + 1)
                    for hp in range(2):
                        # ---- QK: 64x128 row-tiled pair ----
                        psS = {}
                        for hh in range(2):
                            psS[hh] = psum.tile([128, 1024], F32, tag="s",
                                                name=f"psS{hh}")
                        for s_i, j in enumerate(js):
                            # j1 writes full width (masked-anyway scores) so
                            # the [lo,1024) exp below reads no uninit PSUM
                            no = n_off_of(c, j) if s_i == 0 else 0
                            # hh=1 first: its psS slot WAR (exp of h1 two
                            # pairs back) releases later than h0's, so the
                            # second matmul is always ready and the row-tiled
                            # pair overlaps on the PE array
                            for hh in (1, 0):
                                rows = slice(64 * hh, 64 * hh + 64)
                                nc.tensor.matmul(
                                    psS[hh][:, ds(512 * s_i + no, 512 - no)],
                                    kT_sb[hp][rows, ts(j, 128)],
                                    qT_sb[hp][rows, ds(512 * c + no, 512 - no)],
                                    start=True, stop=True,
                                    tile_position=(64 * hh, 0))
                        # ---- exp (+ causal tri / general mask fixups) ----
                        for hh in range(2):
                            h = 2 * hp + hh
                            E = epool.tile([128, 1024], BF16, tag=f"E{h}",
                                           bufs=8, name=f"E{h}")
                            Es[(g, h)] = E
                            lo = n_off_of(c, js[0])
                            nc.scalar.activation(
                                E[:, ds(lo, 1024 - lo)],
                                psS[hh][:, ds(lo, 1024 - lo)],
                                mybir.ActivationFunctionType.Exp, scale=SCALE)
                            for s_i, j in enumerate(js):
                                no = n_off_of(c, j)
                                if mode == "causal" and j >= 4 * c:
                                    nc.vector.tensor_mul(
                                        E[:, ds(512 * s_i + no, 128)],
                                        E[:, ds(512 * s_i + no, 128)],
                                        tri_sb[:])
                                if mode == "general":
                                    nc.vector.tensor_mul(
                                        E[:, ts(s_i, 512)], E[:, ts(s_i, 512)],
                                        msk_sb[:, j, :])
                    # ---- PV: 128x64 col-tiled pairs ----
                    for hp in range(2):
                        for s_i, j in enumerate(js):
                            no = n_off_of(c, j)
                            # hh=1 first: E_h1 is the later-ready operand, so
                            # the second (h0) matmul is always ready and the
                            # col-tiled pair overlaps on the PE array
                            for hh in (1, 0):
                                E = Es[(g, 2 * hp + hh)]
                                nc.tensor.matmul(
                                    psO[hp][64 * hh:64 * hh + 64,
                                            ds(no, 512 - no)],
                                    v_sb[j][:, ds(128 * hp + 64 * hh, 64)],
                                    E[:, ds(512 * s_i + no, 512 - no)],
                                    start=(j == 0), stop=(j == nkt - 1),
                                    tile_position=(0, 64 * hh),
                                    skip_group_check=True)
                    # ---- PE filler work (standard 128x128 mode) ----
                    want = (nfill * (g + 1)) // ngroups
                    while nemit < want:
                        fillers[nemit]()
                        nemit += 1
                while nemit < nfill:
                    fillers[nemit]()
                    nemit += 1

                # ---- softmax denominators: 128x32 col-tiled quads ----
                psD = psum.tile([128, 1024], F32, tag="s", name="psD")
                for j in range(nkt):
                    no = n_off_of(c, j)
                    g, s_i = j // 2, j % 2
                    for h in range(HG):
                        nc.tensor.matmul(
                            psD[32 * h:32 * h + 1, ds(no, 512 - no)],
                            ones_sb[:],
                            Es[(g, h)][:, ds(512 * s_i + no, 512 - no)],
                            start=(j == 0), stop=(j == nkt - 1),
                            tile_position=(0, 32 * h),
                            skip_group_check=True)
                # ---- normalize: o = psO * (1/d) ----
                # d rows live at partitions 32h (same as psD); a K=64
                # row-tiled matmul against blk (ones at local rows 0/32)
                # replicates each head's row across its 64 psO partitions.
                for h in range(HG):
                    nc.vector.tensor_copy(dT_sb[32 * h:32 * h + 1, :],
                                          psD[32 * h:32 * h + 1, 0:512])
                if last:
                    # dummy matmuls keep the HAM clock-gate open through the
                    # vector-only normalize window so the final o-projection
                    # runs at full clock
                    for i in range(16):
                        psW = psum.tile([128, 512], F32, tag="oy", bufs=1,
                                        name="psWt")
                        nc.tensor.matmul(psW[:, 0:512], warm[:],
                                         qT_sb[0][:, 0:512],
                                         start=True, stop=True)
                if dbg and c == 0:
                    for h in range(HG):
                        nc.sync.dma_start(d_dD[32 * h:32 * h + 1, :],
                                          dT_sb[32 * h:32 * h + 1, :])
                    nc.sync.dma_start(d_E[:], Es[(0, 0)][:])
                o_pairs = []
                for hp in range(2):
                    rbp = psum.tile([128, 512], F32, tag="proj", bufs=1,
                                    name="rbp")
                    nc.tensor.matmul(rbp[:], blk_sb[64 * hp:64 * hp + 64, :],
                                     dT_sb[64 * hp:64 * hp + 64, :],
                                     start=True, stop=True,
                                     tile_position=(64 * hp, 0))
                    ri = opool.tile([128, 512], F32, tag="ri", name="ri")
                    nc.vector.reciprocal_approx_fast(ri[:], rbp[:])
                    if dbg and c == 0 and hp == 0:
                        tmp = opool.tile([128, 512], F32, tag="dbgtmp",
                                         name="dbgtmp")
                        nc.vector.tensor_copy(tmp[:], psO[0][:])
                        nc.sync.dma_start(d_psO0[:], tmp[:])
                        nc.sync.dma_start(d_rb0[:], ri[:])
                        nc.sync.dma_start(d_ri0[:], ri[:])
                    o_sb = opool.tile([128, 512], BF16, tag=f"os{hp}",
                                      name=f"o_sb{hp}")
                    nc.vector.tensor_mul(o_sb[:], psO[hp][:], ri[:])
                    o_pairs.append(o_sb)
                if dbg and c == 0:
                    nc.sync.dma_start(d_o0[:], o_pairs[0][:])
                return o_pairs

            # ---------------- schedule ----------------
            # PE warm-up bridges the x-chunk-0 DMA window (~4us) and keeps
            # the HAM activity monitor from clock-gating the first matmuls.
            warm = persist.tile([128, 128], BF16, name="warm")
            nc.vector.memset(warm[:], 0.0)
            for i in range(32):
                psW = psum.tile([128, 512], F32, tag="proj", bufs=1,
                                name="psW")
                nc.tensor.matmul(psW[:, 0:128], warm[:], warm[:],
                                 start=True, stop=True)

            if mode == "causal":
                for part in range(4):
                    emit_qkproj(0, part)
                for tt in range(2):
                    emit_vproj(tt)
                # group 0 of chunk 0 only consumes v0/v1; v2/v3 lead the
                # filler list and land at the group-0 boundary, before the
                # group-1 PV that reads them
                fill = {
                    0: [lambda tt=tt: emit_vproj(tt) for tt in range(2, 8)]
                    + [lambda p=p: emit_qkproj(1, p) for p in range(4)],
                    1: [lambda tt=tt: emit_vproj(tt) for tt in range(8, 12)]
                    + [lambda p=p: emit_qkproj(2, p) for p in range(4)],
                    2: [lambda p=p: emit_qkproj(3, p) for p in range(4)],
                    3: [lambda tt=tt: emit_vproj(tt) for tt in range(12, 16)],
                }
            else:
                for tt in range(KT):
                    emit_vproj(tt)
                for n in range(NCHUNK):
                    for part in range(4):
                        emit_qkproj(n, part)
                fill = {c: [] for c in range(NCHUNK)}

            pending = None
            for c in range(NCHUNK):
                fillers = fill[c]
                if pending is not None:
                    fillers = fillers + [
                        lambda tt=tt, pp=pending: emit_oproj_unit(pp, tt)
                        for tt in range(4)]
                o_pairs = emit_attn(c, fillers, last=(c == NCHUNK - 1))
                pending = (c, o_pairs)
            # tail: cycle psY over the freed o0/o1 banks and split the PSUM
            # evacuation across vector+scalar so matmuls and copies overlap
            tcy = ["oy", "o0", "o1", "oy", "o0", "o1", "oy", "o0"]
            for tt in range(4):
                emit_oproj_unit(pending, tt, tags=tcy[2 * tt:2 * tt + 2],
                                split=True)
            if dbg:
                nc.sync.dma_start(d_qT0[:], qT_sb[0][:])
                nc.sync.dma_start(d_kT0[:], kT_sb[0][:])
                nc.sync.dma_start(d_v0[:], v_sb[0][:])
                nc.sync.dma_start(d_v5[:], v_sb[5][:])

    nc.finalize()
    return nc


_CACHE: dict = {}


def _get_nc(mode: str):
    if mode not in _CACHE:
        _CACHE[mode] = build(mode)
    return _CACHE[mode]


def kernel(x, cos, sin, mask, n_heads, Wq, Wk, Wv, Wo, _trace=False):
    _install_ntff_hook()
    assert int(n_heads) == H, f"kernel hardcodes {H} heads, got {n_heads}"
    x = np.asarray(x, np.float32)
    cos = np.asarray(cos, np.float32)
    sin = np.asarray(sin, np.float32)
    mask = np.asarray(mask)
    Wq = np.asarray(Wq, np.float32)
    Wk = np.asarray(Wk, np.float32)
    Wv = np.asarray(Wv, np.float32)
    Wo = np.asarray(Wo, np.float32)

    if np.array_equal(mask, np.tril(np.ones((T, T), mask.dtype))):
        mode = "causal"
    elif np.all(mask == 1):
        mode = "full"
    else:
        mode = "general"

    cosT = np.ascontiguousarray(cos.T)          # (64, T)
    # pre-swapped signed sin: after multiplying q by this and swapping the
    # 32-row halves, we get rotate_half(q)*sin in standard orientation.
    sinS = np.ascontiguousarray(sin.T).copy()
    sinS[32:64] *= -1.0
    cos2 = np.vstack([cosT, cosT]).astype(ml_dtypes.bfloat16)   # (128, T)
    sin2 = np.vstack([sinS, sinS]).astype(ml_dtypes.bfloat16)
    tri = np.triu(np.ones((128, 128), np.float32)).astype(ml_dtypes.bfloat16)
    blk = np.zeros((128, 128), np.float32)
    blk[[0, 64], 0:64] = 1.0
    blk[[32, 96], 64:128] = 1.0
    blk = blk.astype(ml_dtypes.bfloat16)

    in_maps = []
    for core in range(NCORES):
        b, g = core // 4, core % 4
        rows = slice(g * 256, (g + 1) * 256)
        m = {
            "xT": np.ascontiguousarray(x[b].T).astype(ml_dtypes.bfloat16),
            "wq": np.ascontiguousarray(Wq[rows].T).astype(ml_dtypes.bfloat16),
            "wk": np.ascontiguousarray(Wk[rows].T).astype(ml_dtypes.bfloat16),
            "wv": np.ascontiguousarray(Wv[rows].T).astype(ml_dtypes.bfloat16),
            "wo": np.ascontiguousarray(Wo[:, rows].T).astype(ml_dtypes.bfloat16),
            "cos2": cos2, "sin2": sin2, "tri": tri, "blk": blk,
        }
        if mode == "general":
            m["maskT"] = np.ascontiguousarray(mask.T).astype(ml_dtypes.bfloat16)
        in_maps.append(m)

    nc = _get_nc(mode)
    res = bass_utils.run_bass_kernel_spmd(
        nc, in_maps, core_ids=list(range(NCORES)), trace=_trace)
    if _trace:
        kernel.last_result = res

    y = np.zeros((B, T, C), np.float32)
    for core in range(NCORES):
        y[core // 4] += res.results[core]["out"].astype(np.float32)
    return y
